# revision 14
# baseline (speedup 1.0000x reference)
# Trainium2 Bass kernel for nn_ConditionedCTKoopmanTransition.
#
# Math (reference): z' = z @ A_bar^T + u @ B_bar^T ; y = z' @ C^T + (u*dt) @ D^T
# scanned over T=256 steps, with A_bar = expm(A_ct*dt), B_bar = A^-1 (A_bar-I) B_ct
# built host-side in float64 from the tiny parameter tensors.
#
# Strategy: data-parallel over batch (8 cores x 64 batch). On each core the
# T=256 sequential scan is restructured into 8 chunks of S=32 steps.  The
# chunk-anchor states z_{32k} depend on the inputs only through
#   F_k = [A^31 B | ... | B] @ u-block_k,   a_{k+1} = A^32 a_k + F_k
# which is tiny dense linear algebra -> computed on the HOST in float64.
# The device then rolls all 8 chunks forward simultaneously, batched in the
# matmul free dimension (N = 8 chunks x 64 batch = 512), so every
# tensor-engine op is a full-width [K<=128, M<=128, N=512] fp16 matmul with
# fast weight loads, instead of 256 sequential N=64 steps.  State is kept
# d-major (z^T) so each step's PSUM output feeds the next step's matmul rhs
# directly -- no transposes anywhere on device.  The small K=32 drive
# matmuls (B u_t, D u_t) are packed into distinct 32-row PE sub-array tiles
# (tile_position via base_partition) so they run concurrently.

import sys
import numpy as np

sys.path.insert(0, "/opt/trn_rl_repo")

D = 512
UD = 32
NOBS = 50
BATCH = 512
T = 256
NCORES = 8
BS = BATCH // NCORES      # batch shard per core = 64
S = 32                    # chunk length
NCH = T // S              # chunks = 8
NF = NCH * BS             # matmul free dim = 512

_PROGRAM_CACHE = {}
TRACE = False             # test harness can set kernel.TRACE = True
LAST_RESULT = None        # BassKernelResults of the last run (when TRACE)
MM_DTYPE = "f16"          # "f16" (fast weight load) or "f32r" (highest precision)


def _softplus64(x):
    x = np.asarray(x, np.float64)
    return np.log1p(np.exp(-np.abs(x))) + np.maximum(x, 0.0)


def _host_precompute(dt_val, A_skew_params, gamma_raw, B_ct):
    """float64 host math for the small matrices."""
    import scipy.linalg as sla
    d = D
    A = np.zeros((d, d), np.float64)
    iu = np.triu_indices(d, k=1)
    A[iu] = np.asarray(A_skew_params, np.float64)
    A = A - A.T
    A_ct = A - np.diag(_softplus64(gamma_raw))
    A_bar = sla.expm(A_ct * float(dt_val))
    B_bar = np.linalg.solve(A_ct, (A_bar - np.eye(d)) @ np.asarray(B_ct, np.float64))
    G = np.zeros((d, S * UD), np.float64)
    M = B_bar.copy()
    for j in range(S - 1, -1, -1):
        G[:, j * UD:(j + 1) * UD] = M
        if j > 0:
            M = A_bar @ M
    A_S = np.linalg.matrix_power(A_bar, S)
    return A_bar, B_bar, G, A_S


def _build_program(mm_key):
    from concourse import bacc, tile, mybir

    f32 = mybir.dt.float32
    mdt = {"f16": mybir.dt.float16, "f32r": mybir.dt.float32r}[mm_key]

    nc = bacc.Bacc("TRN2", target_bir_lowering=False, debug=False,
                   num_devices=NCORES)

    # DRAM I/O in the matmul dtype so plain DMAs land in matching tiles.
    wat_d = nc.dram_tensor("wat", [D, D], mdt, kind="ExternalInput")
    wbt_d = nc.dram_tensor("wbt", [96, D], mdt, kind="ExternalInput")
    wct_d = nc.dram_tensor("wct", [D, NOBS], mdt, kind="ExternalInput")
    wdt_d = nc.dram_tensor("wdt", [64, NOBS], mdt, kind="ExternalInput")
    uall_d = nc.dram_tensor("uall", [S * UD, NF], mdt, kind="ExternalInput")
    an0_d = nc.dram_tensor("an0", [D, NF], mdt, kind="ExternalInput")
    ztout_d = nc.dram_tensor("ztout", [NCH, S, D, BS], mdt, kind="ExternalOutput")
    ytout_d = nc.dram_tensor("ytout", [NCH, S, NOBS, BS], f32, kind="ExternalOutput")

    KT = D // 128   # 4 k-tiles of the d dimension

    with tile.TileContext(nc) as tc:
        with tc.tile_pool(name="const", bufs=1) as cpool, \
             tc.tile_pool(name="anp", bufs=1) as anpool, \
             tc.tile_pool(name="st", bufs=3) as stpool, \
             tc.tile_pool(name="ysb", bufs=3) as ypool, \
             tc.tile_pool(name="acc", bufs=6, space="PSUM") as apool, \
             tc.tile_pool(name="yacc", bufs=2, space="PSUM") as yapool:

            # ---- load constants ----
            an = {}
            for m in range(KT):
                an[m] = anpool.tile([128, NF], mdt, tag=f"an{m}", name=f"an{m}")
                nc.sync.dma_start(an[m][:], an0_d.ap()[128 * m:128 * (m + 1), :])
            wa = []
            wc = []
            for kk in range(KT):
                t = cpool.tile([128, D], mdt, tag=f"wa{kk}")
                nc.sync.dma_start(t[:], wat_d.ap()[128 * kk:128 * (kk + 1), :])
                wa.append(t)
                t = cpool.tile([128, NOBS], mdt, tag=f"wc{kk}")
                nc.sync.dma_start(t[:], wct_d.ap()[128 * kk:128 * (kk + 1), :])
                wc.append(t)
            # u for step r of every chunk, replicated at partition offsets
            # 0/32/64 so K=32 drive matmuls can pack 3 PE row-tiles.
            uall = []
            for rr in range(S):
                t = cpool.tile([96, NF], mdt, tag=f"u{rr}", name=f"u{rr}")
                for rep in range(3):
                    nc.sync.dma_start(t[32 * rep:32 * (rep + 1), :],
                                      uall_d.ap()[UD * rr:UD * (rr + 1), :])
                uall.append(t)
            wb = cpool.tile([96, D], mdt, tag="wb")         # B_bar^T replicated 3x
            nc.sync.dma_start(wb[:], wbt_d.ap())
            wd = cpool.tile([64, NOBS], mdt, tag="wd")      # (dt D)^T replicated 2x
            nc.sync.dma_start(wd[:], wdt_d.ap())

            # ---- batched rollout of all chunks (N=512 matmuls) ----
            state = {m: an[m] for m in range(KT)}
            for r in range(S):
                ur = uall[r]
                new = {}
                ps = {}
                for m in range(KT):
                    ps[m] = apool.tile([128, NF], f32, tag="acc", name=f"ps{m}")
                    for kk in range(KT):
                        nc.tensor.matmul(
                            ps[m][:],
                            wa[kk][:, 128 * m:128 * (m + 1)],
                            state[kk][:],
                            start=(kk == 0), stop=False,
                        )
                # drive terms: pack K=32 matmuls into PE row-tiles 0/32/64
                py = yapool.tile([NOBS, NF], f32, tag="yacc")
                for m in range(3):
                    nc.tensor.matmul(
                        ps[m][:],
                        wb[32 * m:32 * (m + 1), 128 * m:128 * (m + 1)],
                        ur[32 * m:32 * (m + 1), :],
                        start=False, stop=True,
                    )
                nc.tensor.matmul(
                    ps[3][:],
                    wb[0:32, 384:512],
                    ur[0:32, :],
                    start=False, stop=True,
                )
                nc.tensor.matmul(
                    py[:], wd[32:64, :], ur[32:64, :],
                    start=True, stop=False,
                )
                for m in range(KT):
                    ns = stpool.tile([128, NF], mdt, tag=f"st{m}", name=f"ns{m}")
                    nc.vector.tensor_copy(ns[:], ps[m][:])
                    new[m] = ns
                    nc.sync.dma_start(
                        ztout_d.ap()[:, r, 128 * m:128 * (m + 1), :]
                        .rearrange("k p e -> p k e"),
                        ns[:].rearrange("p (k e) -> p k e", e=BS),
                    )
                # y = C z' + (dt D) u
                for kk in range(KT):
                    nc.tensor.matmul(
                        py[:], wc[kk][:], new[kk][:],
                        start=False, stop=(kk == KT - 1),
                    )
                yt = ypool.tile([NOBS, NF], f32, tag="y")
                nc.vector.tensor_copy(yt[:], py[:])
                nc.sync.dma_start(
                    ytout_d.ap()[:, r, :, :].rearrange("k p e -> p k e"),
                    yt[:].rearrange("p (k e) -> p k e", e=BS),
                )
                for m in range(KT):
                    state[m] = new[m]

    nc.compile()
    return nc


def _get_program():
    if MM_DTYPE not in _PROGRAM_CACHE:
        _PROGRAM_CACHE[MM_DTYPE] = _build_program(MM_DTYPE)
    return _PROGRAM_CACHE[MM_DTYPE]


def kernel(z_dyn, z_static, dt, U, A_skew_params, gamma_raw, B_ct, C, D_mat=None, **kw):
    # accept the reference's keyword name "D"
    if D_mat is None:
        D_mat = kw.pop("D")
    from concourse import bass_utils

    z_dyn = np.asarray(z_dyn)
    U = np.asarray(U)
    dt_val = float(np.asarray(dt)[0, 0])
    A_bar, B_bar, G, A_S = _host_precompute(dt_val, A_skew_params, gamma_raw, B_ct)

    nc = _get_program()

    mmnp = np.float16 if MM_DTYPE == "f16" else np.float32
    wat = np.ascontiguousarray(A_bar.T).astype(mmnp)
    wbt = np.ascontiguousarray(np.tile(B_bar.T, (3, 1))).astype(mmnp)
    wct = np.ascontiguousarray(np.asarray(C, np.float64).T).astype(mmnp)
    wdt = np.ascontiguousarray(
        np.tile((np.asarray(D_mat, np.float64) * dt_val).T, (2, 1))).astype(mmnp)

    # host-side chunk anchors (float64):
    #   F = G @ u-block ; a_{k+1} = A^S a_k + F_k
    U64 = U.astype(np.float64)
    z64 = z_dyn.astype(np.float64)
    in_maps = []
    for c in range(NCORES):
        Uc = U64[:, BS * c:BS * (c + 1), :]                      # [256, 64, 32]
        # UALL[32*j + ui, 64*k + b] = U[32k + j, 64c + b, ui]
        uallc = np.ascontiguousarray(
            Uc.reshape(NCH, S, BS, UD).transpose(1, 3, 0, 2).reshape(S * UD, NF))
        F = G @ uallc                                            # [D, NF]
        AN = np.empty((D, NF), np.float64)
        AN[:, 0:BS] = z64[BS * c:BS * (c + 1), :].T
        for k in range(NCH - 1):
            AN[:, BS * (k + 1):BS * (k + 2)] = (
                A_S @ AN[:, BS * k:BS * (k + 1)] + F[:, BS * k:BS * (k + 1)])
        m = {"wat": wat, "wbt": wbt, "wct": wct, "wdt": wdt,
             "uall": uallc.astype(mmnp),
             "an0": np.ascontiguousarray(AN).astype(mmnp)}
        in_maps.append(m)

    global LAST_RESULT
    res = bass_utils.run_bass_kernel_spmd(
        nc, in_maps, core_ids=list(range(NCORES)), trace=TRACE,
    )
    LAST_RESULT = res

    Z = np.empty((T, BATCH, D), np.float32)
    Y = np.empty((T, BATCH, NOBS), np.float32)
    for c in range(NCORES):
        zt = res.results[c]["ztout"].astype(np.float32).reshape(T, D, BS)
        yt = res.results[c]["ytout"].reshape(T, NOBS, BS)
        Z[:, BS * c:BS * (c + 1), :] = zt.transpose(0, 2, 1)
        Y[:, BS * c:BS * (c + 1), :] = yt.transpose(0, 2, 1)
    return Z, Y


# revision 15
# speedup vs baseline: 1.2568x; 1.2568x over previous
# Trainium2 Bass kernel for nn_ConditionedCTKoopmanTransition.
#
# Math (reference): z' = z @ A_bar^T + u @ B_bar^T ; y = z' @ C^T + (u*dt) @ D^T
# scanned over T=256 steps, with A_bar = expm(A_ct*dt), B_bar = A^-1 (A_bar-I) B_ct
# built host-side in float64 from the tiny parameter tensors.
#
# Strategy: data-parallel over batch (8 cores x 64 batch). On each core the
# T=256 sequential scan is restructured into 8 chunks of S=32 steps.  The
# chunk-anchor states z_{32k} depend on the inputs only through
#   F_k = [A^31 B | ... | B] @ u-block_k,   a_{k+1} = A^32 a_k + F_k
# which is tiny dense linear algebra -> computed on the HOST in float64.
# The device then rolls all 8 chunks forward simultaneously, batched in the
# matmul free dimension (N = 8 chunks x 64 batch = 512), so every
# tensor-engine op is a full-width [K<=128, M<=128, N=512] fp16 matmul with
# fast weight loads, instead of 256 sequential N=64 steps.  State is kept
# d-major (z^T) so each step's PSUM output feeds the next step's matmul rhs
# directly -- no transposes anywhere on device.  The small K=32 drive
# matmuls (B u_t, D u_t) are packed into distinct 32-row PE sub-array tiles
# (tile_position via base_partition) so they run concurrently.

import sys
import numpy as np

sys.path.insert(0, "/opt/trn_rl_repo")

D = 512
UD = 32
NOBS = 50
BATCH = 512
T = 256
NCORES = 8
BS = BATCH // NCORES      # batch shard per core = 64
S = 32                    # chunk length
NCH = T // S              # chunks = 8
NF = NCH * BS             # matmul free dim = 512

_PROGRAM_CACHE = {}
TRACE = False             # test harness can set kernel.TRACE = True
LAST_RESULT = None        # BassKernelResults of the last run (when TRACE)
MM_DTYPE = "f16"          # "f16" (fast weight load) or "f32r" (highest precision)


def _softplus64(x):
    x = np.asarray(x, np.float64)
    return np.log1p(np.exp(-np.abs(x))) + np.maximum(x, 0.0)


def _host_precompute(dt_val, A_skew_params, gamma_raw, B_ct):
    """float64 host math for the small matrices."""
    import scipy.linalg as sla
    d = D
    A = np.zeros((d, d), np.float64)
    iu = np.triu_indices(d, k=1)
    A[iu] = np.asarray(A_skew_params, np.float64)
    A = A - A.T
    A_ct = A - np.diag(_softplus64(gamma_raw))
    A_bar = sla.expm(A_ct * float(dt_val))
    B_bar = np.linalg.solve(A_ct, (A_bar - np.eye(d)) @ np.asarray(B_ct, np.float64))
    G = np.zeros((d, S * UD), np.float64)
    M = B_bar.copy()
    for j in range(S - 1, -1, -1):
        G[:, j * UD:(j + 1) * UD] = M
        if j > 0:
            M = A_bar @ M
    A_S = np.linalg.matrix_power(A_bar, S)
    return A_bar, B_bar, G, A_S


def _build_program(mm_key):
    from concourse import bacc, tile, mybir

    f32 = mybir.dt.float32
    mdt = {"f16": mybir.dt.float16, "f32r": mybir.dt.float32r}[mm_key]

    nc = bacc.Bacc("TRN2", target_bir_lowering=False, debug=False,
                   num_devices=NCORES)

    # DRAM I/O in the matmul dtype so plain DMAs land in matching tiles.
    wat_d = nc.dram_tensor("wat", [D, D], mdt, kind="ExternalInput")
    wbt_d = nc.dram_tensor("wbt", [4 * 128, D], mdt, kind="ExternalInput")
    wct_d = nc.dram_tensor("wct", [D, NOBS], mdt, kind="ExternalInput")
    wdt_d = nc.dram_tensor("wdt", [4 * 128, NOBS], mdt, kind="ExternalInput")
    uall_d = nc.dram_tensor("uall", [S * UD, NF], mdt, kind="ExternalInput")
    an0_d = nc.dram_tensor("an0", [D, NF], mdt, kind="ExternalInput")
    ztout_d = nc.dram_tensor("ztout", [NCH, S, D, BS], mdt, kind="ExternalOutput")
    ytout_d = nc.dram_tensor("ytout", [NCH, S, NOBS, BS], f32, kind="ExternalOutput")

    KT = D // 128   # 4 k-tiles of the d dimension

    with tile.TileContext(nc) as tc:
        with tc.tile_pool(name="const", bufs=1) as cpool, \
             tc.tile_pool(name="anp", bufs=1) as anpool, \
             tc.tile_pool(name="st", bufs=3) as stpool, \
             tc.tile_pool(name="ysb", bufs=3) as ypool, \
             tc.tile_pool(name="acc", bufs=6, space="PSUM") as apool, \
             tc.tile_pool(name="yacc", bufs=2, space="PSUM") as yapool:

            # ---- load constants ----
            an = {}
            for m in range(KT):
                an[m] = anpool.tile([128, NF], mdt, tag=f"an{m}", name=f"an{m}")
                nc.sync.dma_start(an[m][:], an0_d.ap()[128 * m:128 * (m + 1), :])
            wa = []
            wc = []
            for kk in range(KT):
                t = cpool.tile([128, D], mdt, tag=f"wa{kk}")
                nc.sync.dma_start(t[:], wat_d.ap()[128 * kk:128 * (kk + 1), :])
                wa.append(t)
                t = cpool.tile([128, NOBS], mdt, tag=f"wc{kk}")
                nc.sync.dma_start(t[:], wct_d.ap()[128 * kk:128 * (kk + 1), :])
                wc.append(t)
            # u in 8 full-partition tiles (4 steps per tile, rows 32q:32q+32)
            ubig = []
            for kk in range(S * UD // 128):
                t = cpool.tile([128, NF], mdt, tag=f"ub{kk}", name=f"ub{kk}")
                nc.sync.dma_start(t[:], uall_d.ap()[128 * kk:128 * (kk + 1), :])
                ubig.append(t)
            # drive weights zero-padded to K=128, one variant per r%4 with
            # B_bar^T / (dt D)^T placed at rows 32q:32q+32 -- so the drive
            # matmul consumes the full u tile with no partition offsets.
            wb = []
            wd = []
            for qq in range(4):
                t = cpool.tile([128, D], mdt, tag=f"wb{qq}", name=f"wb{qq}")
                nc.sync.dma_start(t[:], wbt_d.ap()[128 * qq:128 * (qq + 1), :])
                wb.append(t)
                t = cpool.tile([128, NOBS], mdt, tag=f"wd{qq}", name=f"wd{qq}")
                nc.sync.dma_start(t[:], wdt_d.ap()[128 * qq:128 * (qq + 1), :])
                wd.append(t)

            # ---- batched rollout of all chunks (N=512 matmuls) ----
            state = {m: an[m] for m in range(KT)}
            for r in range(S):
                ur = ubig[r // 4]
                q = r % 4
                new = {}
                ps = {}
                for m in range(KT):
                    ps[m] = apool.tile([128, NF], f32, tag="acc", name=f"ps{m}")
                    for kk in range(KT):
                        nc.tensor.matmul(
                            ps[m][:],
                            wa[kk][:, 128 * m:128 * (m + 1)],
                            state[kk][:],
                            start=(kk == 0), stop=False,
                        )
                py = yapool.tile([NOBS, NF], f32, tag="yacc")
                for m in range(KT):
                    nc.tensor.matmul(
                        ps[m][:],
                        wb[q][:, 128 * m:128 * (m + 1)],
                        ur[:],
                        start=False, stop=True,
                    )
                nc.tensor.matmul(
                    py[:], wd[q][:], ur[:],
                    start=True, stop=False,
                )
                for m in range(KT):
                    ns = stpool.tile([128, NF], mdt, tag=f"st{m}", name=f"ns{m}")
                    nc.vector.tensor_copy(ns[:], ps[m][:])
                    new[m] = ns
                    nc.sync.dma_start(
                        ztout_d.ap()[:, r, 128 * m:128 * (m + 1), :]
                        .rearrange("k p e -> p k e"),
                        ns[:].rearrange("p (k e) -> p k e", e=BS),
                    )
                # y = C z' + (dt D) u
                for kk in range(KT):
                    nc.tensor.matmul(
                        py[:], wc[kk][:], new[kk][:],
                        start=False, stop=(kk == KT - 1),
                    )
                yt = ypool.tile([NOBS, NF], f32, tag="y")
                nc.vector.tensor_copy(yt[:], py[:])
                nc.sync.dma_start(
                    ytout_d.ap()[:, r, :, :].rearrange("k p e -> p k e"),
                    yt[:].rearrange("p (k e) -> p k e", e=BS),
                )
                for m in range(KT):
                    state[m] = new[m]

    nc.compile()
    return nc


def _get_program():
    if MM_DTYPE not in _PROGRAM_CACHE:
        _PROGRAM_CACHE[MM_DTYPE] = _build_program(MM_DTYPE)
    return _PROGRAM_CACHE[MM_DTYPE]


def kernel(z_dyn, z_static, dt, U, A_skew_params, gamma_raw, B_ct, C, D_mat=None, **kw):
    # accept the reference's keyword name "D"
    if D_mat is None:
        D_mat = kw.pop("D")
    from concourse import bass_utils

    z_dyn = np.asarray(z_dyn)
    U = np.asarray(U)
    dt_val = float(np.asarray(dt)[0, 0])
    A_bar, B_bar, G, A_S = _host_precompute(dt_val, A_skew_params, gamma_raw, B_ct)

    nc = _get_program()

    mmnp = np.float16 if MM_DTYPE == "f16" else np.float32
    wat = np.ascontiguousarray(A_bar.T).astype(mmnp)
    wct = np.ascontiguousarray(np.asarray(C, np.float64).T).astype(mmnp)
    wbt = np.zeros((4 * 128, D), np.float64)
    wdt = np.zeros((4 * 128, NOBS), np.float64)
    for qq in range(4):
        wbt[128 * qq + 32 * qq:128 * qq + 32 * (qq + 1), :] = B_bar.T
        wdt[128 * qq + 32 * qq:128 * qq + 32 * (qq + 1), :] = (
            np.asarray(D_mat, np.float64) * dt_val).T
    wbt = np.ascontiguousarray(wbt).astype(mmnp)
    wdt = np.ascontiguousarray(wdt).astype(mmnp)

    # host-side chunk anchors (float64):
    #   F = G @ u-block ; a_{k+1} = A^S a_k + F_k
    U64 = U.astype(np.float64)
    z64 = z_dyn.astype(np.float64)
    in_maps = []
    for c in range(NCORES):
        Uc = U64[:, BS * c:BS * (c + 1), :]                      # [256, 64, 32]
        # UALL[32*j + ui, 64*k + b] = U[32k + j, 64c + b, ui]
        uallc = np.ascontiguousarray(
            Uc.reshape(NCH, S, BS, UD).transpose(1, 3, 0, 2).reshape(S * UD, NF))
        F = G @ uallc                                            # [D, NF]
        AN = np.empty((D, NF), np.float64)
        AN[:, 0:BS] = z64[BS * c:BS * (c + 1), :].T
        for k in range(NCH - 1):
            AN[:, BS * (k + 1):BS * (k + 2)] = (
                A_S @ AN[:, BS * k:BS * (k + 1)] + F[:, BS * k:BS * (k + 1)])
        m = {"wat": wat, "wbt": wbt, "wct": wct, "wdt": wdt,
             "uall": uallc.astype(mmnp),
             "an0": np.ascontiguousarray(AN).astype(mmnp)}
        in_maps.append(m)

    global LAST_RESULT
    res = bass_utils.run_bass_kernel_spmd(
        nc, in_maps, core_ids=list(range(NCORES)), trace=TRACE,
    )
    LAST_RESULT = res

    Z = np.empty((T, BATCH, D), np.float32)
    Y = np.empty((T, BATCH, NOBS), np.float32)
    for c in range(NCORES):
        zt = res.results[c]["ztout"].astype(np.float32).reshape(T, D, BS)
        yt = res.results[c]["ytout"].reshape(T, NOBS, BS)
        Z[:, BS * c:BS * (c + 1), :] = zt.transpose(0, 2, 1)
        Y[:, BS * c:BS * (c + 1), :] = yt.transpose(0, 2, 1)
    return Z, Y
